# revision 8
# baseline (speedup 1.0000x reference)
"""MoE expert-gate kernel for Trainium2 (8 NeuronCores, SPMD).

Computes, for hidden_states [4, 4096, 4096] f32 and gate_weight [64, 4096] f32:
  logits      = hidden @ gate_weight.T            [B,S,64]
  probs       = softmax(logits, -1)
  (w, idx)    = top2(probs), renormalized w
  loss        = sum(expert_weights * expert_counts) / (T*K/64)^2

Sharding: tokens (B*S = 16384) split evenly across 8 cores (2048 each);
gate weight replicated.  Per-expert weight/count partial histograms are
returned per core and reduced on the host (cheaper than an on-device
all-reduce of 512 B).

Per-core dataflow (T=2048 tokens, H=4096, E=64):
  x [T,H] --DMA--> SBUF [128,4096] tiles
     --PE transpose (128x128 blocks)--> PSUM --copy--> SBUF xT [128H, 512T]
  gate W^T chunks [128,64] stationary; xT moving (N=512), 2-way column
  tiling on the PE array -> PSUM logits [2*64E, 512T]
     --copy--> SBUF --PE transpose (accumulating halves)--> logits [128T, 64E]
  softmax / top-2 / histogram on DVE+ACT, outputs DMA'd per 128-token tile.
"""

from contextlib import ExitStack

import numpy as np

import concourse.bass as bass
import concourse.bacc as bacc
import concourse.tile as tile
from concourse import mybir
from concourse.bass_utils import run_bass_kernel_spmd
from concourse.masks import make_identity

F32 = mybir.dt.float32
U32 = mybir.dt.uint32
AX = mybir.AxisListType
OP = mybir.AluOpType
AF = mybir.ActivationFunctionType

N_CORES = 8
B, S = 4, 4096
H = 4096
E = 64
TOPK = 2
T_TOTAL = B * S                 # 16384
T_CORE = T_TOTAL // N_CORES     # 2048
P = 128
NH = H // P                     # 32 contraction chunks
G = 512                         # tokens per matmul group (PE moving dim)
NG = T_CORE // G                # 4
NT = G // P                     # 4 token tiles per group
N_TILES = T_CORE // P           # 16

EXPECTED_LOAD = float(T_TOTAL * TOPK) / E   # 512.0


def _emit(tc, x_d, wt_d, probs_d, topw_d, topi_d, hist_d):
    nc = tc.nc
    with ExitStack() as ctx:
        const_pool = ctx.enter_context(tc.tile_pool(name="const", bufs=1))
        x_pool = ctx.enter_context(tc.tile_pool(name="x", bufs=8))
        xT_pool = ctx.enter_context(tc.tile_pool(name="xT", bufs=6))
        lg_pool = ctx.enter_context(tc.tile_pool(name="lg", bufs=2))
        lt_pool = ctx.enter_context(tc.tile_pool(name="lt", bufs=6))
        post_pool = ctx.enter_context(tc.tile_pool(name="post", bufs=4))
        out_pool = ctx.enter_context(tc.tile_pool(name="outs", bufs=4))
        tr_psum = ctx.enter_context(tc.tile_pool(name="trps", bufs=3, space="PSUM"))
        lg_psum = ctx.enter_context(tc.tile_pool(name="lgps", bufs=2, space="PSUM"))
        fl_psum = ctx.enter_context(tc.tile_pool(name="flps", bufs=2, space="PSUM"))
        hist_psum = ctx.enter_context(tc.tile_pool(name="hps", bufs=1, space="PSUM"))

        ident = const_pool.tile([P, P], F32)
        make_identity(nc, ident)
        ones = const_pool.tile([P, 1], F32)
        nc.gpsimd.memset(ones, 1.0)

        wt_sb = const_pool.tile([P, NH * E], F32)
        nc.sync.dma_start(out=wt_sb, in_=wt_d)

        hist_ps = hist_psum.tile([1, 2 * E], F32)

        for g in range(NG):
            # ---- load 4 x tiles (2 MiB each, contiguous rows) ----
            xg = []
            for t in range(NT):
                xt = x_pool.tile([P, H], F32, tag="x")
                r0 = (g * NT + t) * P
                nc.sync.dma_start(out=xt, in_=x_d[r0:r0 + P, :])
                xg.append(xt)

            # ---- transpose x tiles: [128T,128H] blocks -> xT [128H, 512T] ----
            xTs = []
            for h in range(NH):
                ps = tr_psum.tile([P, G], F32, tag="tr")
                for t in range(NT):
                    nc.tensor.transpose(
                        ps[:, t * P:(t + 1) * P],
                        xg[t][:, h * P:(h + 1) * P],
                        ident,
                    )
                xT = xT_pool.tile([P, G], F32, tag="xT")
                # split the PSUM->SBUF drain between DVE and ACT
                if h % 32 < 18:
                    nc.scalar.copy(out=xT, in_=ps)
                else:
                    nc.vector.tensor_copy(out=xT, in_=ps)
                xTs.append(xT)

            # ---- gating matmul, 2-way column tiling ----
            # psum [0:64]  accumulates even H-chunks, [64:128] odd chunks
            lgp = lg_psum.tile([P, G], F32, tag="lg")
            for hp in range(NH // 2):
                first, last = hp == 0, hp == NH // 2 - 1
                # The two column-tile halves accumulate into disjoint
                # partition ranges of one PSUM bank; the sim's zero-region
                # group check is partition-blind, so it must be skipped.
                nc.tensor.matmul(
                    lgp[0:E, :],
                    lhsT=wt_sb[:, (2 * hp) * E:(2 * hp + 1) * E],
                    rhs=xTs[2 * hp],
                    start=first, stop=last, skip_group_check=True,
                )
                nc.tensor.matmul(
                    lgp[E:2 * E, :],
                    lhsT=wt_sb[:, (2 * hp + 1) * E:(2 * hp + 2) * E],
                    rhs=xTs[2 * hp + 1],
                    start=first, stop=last, skip_group_check=True,
                )

            up = lg_pool.tile([E, G], F32, tag="up")
            dn = lg_pool.tile([E, G], F32, tag="dn")
            nc.vector.tensor_copy(out=up, in_=lgp[0:E, :])
            nc.vector.tensor_copy(out=dn, in_=lgp[E:2 * E, :])

            for t in range(NT):
                tile_idx = g * NT + t
                r0 = tile_idx * P

                # ---- flip logits to [128T, 64E]; transpose-accumulate adds
                # the two column-tile halves for free ----
                flp = fl_psum.tile([P, E], F32, tag="fl")
                nc.tensor.matmul(
                    flp, lhsT=up[:, t * P:(t + 1) * P], rhs=ident[0:E, 0:E],
                    is_transpose=True, start=True, stop=False,
                )
                nc.tensor.matmul(
                    flp, lhsT=dn[:, t * P:(t + 1) * P], rhs=ident[0:E, 0:E],
                    is_transpose=True, start=False, stop=True,
                )
                lt = lt_pool.tile([P, E], F32, tag="lt")
                nc.vector.tensor_copy(out=lt, in_=flp)

                # ---- softmax / top-2 ----
                m8 = post_pool.tile([P, 8], F32, tag="m8")
                nc.vector.max(out=m8, in_=lt)
                i8 = post_pool.tile([P, 8], U32, tag="i8")
                nc.vector.max_index(out=i8, in_max=m8, in_values=lt)

                negm = post_pool.tile([P, 1], F32, tag="negm")
                nc.vector.tensor_scalar_mul(negm, m8[:, 0:1], -1.0)

                et = post_pool.tile([P, E], F32, tag="E")
                zt = post_pool.tile([P, 1], F32, tag="Z")
                # et = exp(l - m1), zt = row sum of et
                nc.scalar.activation(et, lt, AF.Exp, bias=negm, scale=1.0,
                                     accum_out=zt)
                invz = post_pool.tile([P, 1], F32, tag="invz")
                nc.vector.reciprocal(invz, zt)

                probs_t = out_pool.tile([P, E], F32, tag="probs")
                nc.scalar.activation(probs_t, et, AF.Copy, scale=invz)
                nc.scalar.dma_start(out=probs_d[r0:r0 + P, :], in_=probs_t)

                # renormalized top-2 weights: w_k = (e_k/Z) / ((e1+e2)/Z + 1e-8)
                e2 = post_pool.tile([P, TOPK], F32, tag="e2")
                nc.scalar.activation(e2, m8[:, 0:TOPK], AF.Exp, bias=negm,
                                     scale=1.0)
                s2 = post_pool.tile([P, 1], F32, tag="s2")
                nc.vector.reduce_sum(s2, e2, axis=AX.X)
                sp = post_pool.tile([P, 1], F32, tag="sp")
                nc.vector.tensor_scalar(sp, s2, invz, 1e-8,
                                        op0=OP.mult, op1=OP.add)
                rs = post_pool.tile([P, 1], F32, tag="rs")
                nc.vector.reciprocal(rs, sp)
                c1 = post_pool.tile([P, 1], F32, tag="c1")
                nc.vector.tensor_mul(c1, invz, rs)

                wout = out_pool.tile([P, TOPK], F32, tag="w")
                nc.vector.tensor_scalar_mul(wout, e2, c1)
                nc.scalar.dma_start(out=topw_d[r0:r0 + P, :], in_=wout)
                nc.scalar.dma_start(out=topi_d[r0:r0 + P, :], in_=i8[:, 0:TOPK])

                # ---- load-balance histogram: [weighted | counts] ----
                hm = post_pool.tile([P, 2 * E], F32, tag="hm")
                # mask: 1.0 where e^(l-m1) >= e^(m2-m1)  (== top-2 of the row)
                nc.vector.tensor_scalar(hm[:, E:2 * E], et, e2[:, 1:2], None,
                                        op0=OP.is_ge)
                nc.vector.tensor_scalar_mul(hm[:, 0:E], et, c1)
                nc.vector.tensor_mul(hm[:, 0:E], hm[:, 0:E], hm[:, E:2 * E])
                nc.tensor.matmul(
                    hist_ps, lhsT=ones, rhs=hm,
                    start=(tile_idx == 0), stop=(tile_idx == N_TILES - 1),
                )

        hist_sb = out_pool.tile([1, 2 * E], F32, tag="hist")
        nc.vector.tensor_copy(out=hist_sb, in_=hist_ps)
        nc.scalar.dma_start(out=hist_d, in_=hist_sb)


def build_program():
    nc = bacc.Bacc(
        "TRN2", target_bir_lowering=False, debug=False, num_devices=N_CORES
    )
    x_d = nc.dram_tensor("x", [T_CORE, H], F32, kind="ExternalInput").ap()
    wt_d = nc.dram_tensor("wt", [P, NH * E], F32, kind="ExternalInput").ap()
    probs_d = nc.dram_tensor("probs", [T_CORE, E], F32, kind="ExternalOutput").ap()
    topw_d = nc.dram_tensor("topw", [T_CORE, TOPK], F32, kind="ExternalOutput").ap()
    topi_d = nc.dram_tensor("topi", [T_CORE, TOPK], U32, kind="ExternalOutput").ap()
    hist_d = nc.dram_tensor("hist", [1, 2 * E], F32, kind="ExternalOutput").ap()

    with tile.TileContext(nc) as tc:
        _emit(tc, x_d, wt_d, probs_d, topw_d, topi_d, hist_d)
    # Bacc compile legalizes sync waits (>=2 waits per instruction are split
    # into InstEventSemaphore; walrus only encodes one wait per TPB inst).
    nc.compile()
    return nc


def shard_inputs(hidden_states, gate_weight):
    """Build per-core input maps from the full inputs."""
    x = np.ascontiguousarray(
        np.asarray(hidden_states, dtype=np.float32).reshape(T_TOTAL, H)
    )
    w = np.asarray(gate_weight, dtype=np.float32)
    # W^T in H-chunk-blocked layout: wtb[p, c*E + e] = W[e, c*128 + p]
    wtb = np.ascontiguousarray(
        w.T.reshape(NH, P, E).transpose(1, 0, 2).reshape(P, NH * E)
    )
    in_maps = []
    for c in range(N_CORES):
        shard = np.ascontiguousarray(x[c * T_CORE:(c + 1) * T_CORE])
        in_maps.append({"x": shard, "wt": wtb})
    return in_maps


def assemble_outputs(results):
    """Combine per-core result dicts into the reference's output structure."""
    probs = np.concatenate([r["probs"] for r in results], axis=0)
    topw = np.concatenate([r["topw"] for r in results], axis=0)
    topi = np.concatenate([r["topi"] for r in results], axis=0)
    hist = np.sum(np.stack([r["hist"][0] for r in results]).astype(np.float32),
                  axis=0, dtype=np.float32)
    ew = hist[:E]
    ec = hist[E:]
    loss = np.float32(np.sum(ew * ec, dtype=np.float32) / (EXPECTED_LOAD ** 2))

    top_k_weights = topw.reshape(B, S, TOPK)
    routing_probs = probs.reshape(B, S, E)
    top_k_indices = topi.view(np.int32).reshape(B, S, TOPK)
    return top_k_weights, routing_probs, loss, top_k_indices


_PROGRAM = None


def _get_program():
    global _PROGRAM
    if _PROGRAM is None:
        _PROGRAM = build_program()
    return _PROGRAM


def run(hidden_states, gate_weight, trace=False):
    nc = _get_program()
    in_maps = shard_inputs(hidden_states, gate_weight)
    res = run_bass_kernel_spmd(
        nc, in_maps, list(range(N_CORES)), trace=trace
    )
    return assemble_outputs(res.results), res


def kernel(hidden_states, gate_weight):
    outputs, _ = run(hidden_states, gate_weight)
    return outputs


# revision 16
# speedup vs baseline: 23.4653x; 23.4653x over previous
"""MoE expert-gate kernel for Trainium2 (8 NeuronCores, SPMD).

Computes, for hidden_states [4, 4096, 4096] f32 and gate_weight [64, 4096] f32:
  logits      = hidden @ gate_weight.T            [B,S,64]
  probs       = softmax(logits, -1)
  (w, idx)    = top2(probs), renormalized w
  loss        = sum(expert_weights * expert_counts) / (T*K/64)^2

Sharding: tokens (B*S = 16384) split evenly across 8 cores (2048 each);
gate weight replicated.  Per-expert weight/count partial histograms are
returned per core and reduced on the host (cheaper than an on-device
all-reduce of 512 B).

Per-core dataflow (T=2048 tokens, H=4096, E=64), per 512-token group:
  x [T,H] --DMA--> SBUF [128,4096] tiles
     --PE transpose (128x128 blocks, fp32 is exact)--> PSUM
     --DVE/ACT copy--> SBUF xT [128H, 512T]  (all 32 H-chunks first)
  then a contiguous burst of fp32 matmuls: gate W^T chunks [128,64]
  stationary, xT moving (N=512), 2-way column tiling (tile_position (0,0)
  and (0,64)) -> PSUM logits [2*64E, 512T]; the burst keeps the PE HAM
  clock-gate warm.
     --copy--> SBUF --PE transpose-accumulate--> logits [128T, 64E]
     (the accumulate sums the two column-tile halves for free)
  softmax / top-2 (Max8 + MaxIndex) / renorm / histogram on DVE+ACT,
  outputs DMA'd per 128-token tile on the ACT HWDGE ring (inputs use SP's).

float32r would be ~4x faster for the matmuls but measures ~1.5e-4 rel err
on HW (TF32-grade) — enough to flip top-2 indices; everything stays fp32.
"""

from contextlib import ExitStack

import numpy as np

import concourse.bass as bass
import concourse.bacc as bacc
import concourse.tile as tile
from concourse import mybir
from concourse.bass_utils import run_bass_kernel_spmd
from concourse.masks import make_identity

F32 = mybir.dt.float32
U32 = mybir.dt.uint32
AX = mybir.AxisListType
OP = mybir.AluOpType
AF = mybir.ActivationFunctionType

N_CORES = 8
B, S = 4, 4096
H = 4096
E = 64
TOPK = 2
T_TOTAL = B * S                 # 16384
T_CORE = T_TOTAL // N_CORES     # 2048
P = 128
NH = H // P                     # 32 contraction chunks
G = 512                         # tokens per matmul group (PE moving dim)
NG = T_CORE // G                # 4
NT = G // P                     # 4 token tiles per group
N_TILES = T_CORE // P           # 16

EXPECTED_LOAD = float(T_TOTAL * TOPK) / E   # 512.0


def _emit(tc, x_d, wt_d, probs_d, topw_d, topi_d, hist_d, repeats=1,
          stage="full", burst=True):
    # stage: "full" | "dma" (loads only) | "tr" (loads+transposes+copies)
    nc = tc.nc
    with ExitStack() as ctx:
        const_pool = ctx.enter_context(tc.tile_pool(name="const", bufs=1))
        x_pool = ctx.enter_context(tc.tile_pool(name="x", bufs=5 if burst else 8))
        xT_pool = ctx.enter_context(
            tc.tile_pool(name="xT", bufs=NH + 2 if burst else 6))
        lg_pool = ctx.enter_context(tc.tile_pool(name="lg", bufs=2))
        lt_pool = ctx.enter_context(tc.tile_pool(name="lt", bufs=6))
        post_pool = ctx.enter_context(tc.tile_pool(name="post", bufs=4))
        out_pool = ctx.enter_context(tc.tile_pool(name="outs", bufs=4))
        tr_psum = ctx.enter_context(tc.tile_pool(name="trps", bufs=3, space="PSUM"))
        lg_psum = ctx.enter_context(tc.tile_pool(name="lgps", bufs=2, space="PSUM"))
        fl_psum = ctx.enter_context(tc.tile_pool(name="flps", bufs=2, space="PSUM"))
        hist_psum = ctx.enter_context(tc.tile_pool(name="hps", bufs=1, space="PSUM"))

        ident = const_pool.tile([P, P], F32)
        make_identity(nc, ident)
        ones = const_pool.tile([P, 1], F32)
        nc.gpsimd.memset(ones, 1.0)

        wt_sb = const_pool.tile([P, NH * E], F32)
        nc.sync.dma_start(out=wt_sb, in_=wt_d)

        if stage == "full":
            hist_ps = hist_psum.tile([1, 2 * E], F32, tag="hist_ps")
        else:
            hist_ps = None

        for rep in range(repeats):
         for g in range(NG):
            # ---- load 4 x tiles (2 MiB each, contiguous rows) ----
            xg = []
            for t in range(NT):
                xt = x_pool.tile([P, H], F32, tag="x")
                r0 = (g * NT + t) * P
                nc.sync.dma_start(out=xt, in_=x_d[r0:r0 + P, :])
                xg.append(xt)
            if stage == "dma":
                # keep a data dependency so the loads can't be elided
                probe = xT_pool.tile([P, G], F32, tag="xT")
                nc.vector.tensor_copy(out=probe[:, 0:4],
                                      in_=xg[0][:, 0:4])
                continue

            # ---- transpose x tiles: [128T,128H] blocks -> xT [128H, 512T] ----
            xTs = []
            for h in range(NH):
                ps = tr_psum.tile([P, G], F32, tag="tr")
                for t in range(NT):
                    nc.tensor.transpose(
                        ps[:, t * P:(t + 1) * P],
                        xg[t][:, h * P:(h + 1) * P],
                        ident,
                    )
                xT = xT_pool.tile([P, G], F32, tag="xT")
                # split the PSUM->SBUF drain between DVE and ACT
                if h % 32 < 18:
                    nc.scalar.copy(out=xT, in_=ps)
                else:
                    nc.vector.tensor_copy(out=xT, in_=ps)
                xTs.append(xT)
            if stage == "tr":
                continue

            # ---- gating matmul, 2-way column tiling ----
            # psum [0:64]  accumulates even H-chunks, [64:128] odd chunks
            lgp = lg_psum.tile([P, G], F32, tag="lg")
            for hp in range(NH // 2):
                first, last = hp == 0, hp == NH // 2 - 1
                # The two column-tile halves accumulate into disjoint
                # partition ranges of one PSUM bank; the sim's zero-region
                # group check is partition-blind, so it must be skipped.
                nc.tensor.matmul(
                    lgp[0:E, :],
                    lhsT=wt_sb[:, (2 * hp) * E:(2 * hp + 1) * E],
                    rhs=xTs[2 * hp],
                    start=first, stop=last, skip_group_check=True,
                )
                nc.tensor.matmul(
                    lgp[E:2 * E, :],
                    lhsT=wt_sb[:, (2 * hp + 1) * E:(2 * hp + 2) * E],
                    rhs=xTs[2 * hp + 1],
                    start=first, stop=last, skip_group_check=True,
                )

            up = lg_pool.tile([E, G], F32, tag="up")
            dn = lg_pool.tile([E, G], F32, tag="dn")
            nc.vector.tensor_copy(out=up, in_=lgp[0:E, :])
            nc.vector.tensor_copy(out=dn, in_=lgp[E:2 * E, :])

            for t in range(NT):
                tile_idx = g * NT + t
                r0 = tile_idx * P

                # ---- flip logits to [128T, 64E]; transpose-accumulate adds
                # the two column-tile halves for free ----
                flp = fl_psum.tile([P, E], F32, tag="fl")
                nc.tensor.matmul(
                    flp, lhsT=up[:, t * P:(t + 1) * P], rhs=ident[0:E, 0:E],
                    is_transpose=True, start=True, stop=False,
                )
                nc.tensor.matmul(
                    flp, lhsT=dn[:, t * P:(t + 1) * P], rhs=ident[0:E, 0:E],
                    is_transpose=True, start=False, stop=True,
                )
                lt = lt_pool.tile([P, E], F32, tag="lt")
                nc.vector.tensor_copy(out=lt, in_=flp)

                # ---- softmax / top-2 ----
                m8 = post_pool.tile([P, 8], F32, tag="m8")
                nc.vector.max(out=m8, in_=lt)
                i8 = post_pool.tile([P, 8], U32, tag="i8")
                nc.vector.max_index(out=i8, in_max=m8, in_values=lt)

                negm = post_pool.tile([P, 1], F32, tag="negm")
                nc.vector.tensor_scalar_mul(negm, m8[:, 0:1], -1.0)

                et = post_pool.tile([P, E], F32, tag="E")
                zt = post_pool.tile([P, 1], F32, tag="Z")
                # et = exp(l - m1), zt = row sum of et
                nc.scalar.activation(et, lt, AF.Exp, bias=negm, scale=1.0,
                                     accum_out=zt)
                invz = post_pool.tile([P, 1], F32, tag="invz")
                nc.vector.reciprocal(invz, zt)

                probs_t = out_pool.tile([P, E], F32, tag="probs")
                nc.scalar.activation(probs_t, et, AF.Copy, scale=invz)
                nc.scalar.dma_start(out=probs_d[r0:r0 + P, :], in_=probs_t)

                # renormalized top-2 weights: w_k = (e_k/Z) / ((e1+e2)/Z + 1e-8)
                e2 = post_pool.tile([P, TOPK], F32, tag="e2")
                nc.scalar.activation(e2, m8[:, 0:TOPK], AF.Exp, bias=negm,
                                     scale=1.0)
                s2 = post_pool.tile([P, 1], F32, tag="s2")
                nc.vector.reduce_sum(s2, e2, axis=AX.X)
                sp = post_pool.tile([P, 1], F32, tag="sp")
                nc.vector.tensor_scalar(sp, s2, invz, 1e-8,
                                        op0=OP.mult, op1=OP.add)
                rs = post_pool.tile([P, 1], F32, tag="rs")
                nc.vector.reciprocal(rs, sp)
                c1 = post_pool.tile([P, 1], F32, tag="c1")
                nc.vector.tensor_mul(c1, invz, rs)

                wout = out_pool.tile([P, TOPK], F32, tag="w")
                nc.vector.tensor_scalar_mul(wout, e2, c1)
                nc.scalar.dma_start(out=topw_d[r0:r0 + P, :], in_=wout)
                nc.scalar.dma_start(out=topi_d[r0:r0 + P, :], in_=i8[:, 0:TOPK])

                # ---- load-balance histogram: [weighted | counts] ----
                hm = post_pool.tile([P, 2 * E], F32, tag="hm")
                # mask: 1.0 where e^(l-m1) >= e^(m2-m1)  (== top-2 of the row)
                nc.vector.tensor_scalar(hm[:, E:2 * E], et, e2[:, 1:2], None,
                                        op0=OP.is_ge)
                nc.vector.tensor_scalar_mul(hm[:, 0:E], et, c1)
                nc.vector.tensor_mul(hm[:, 0:E], hm[:, 0:E], hm[:, E:2 * E])
                nc.tensor.matmul(
                    hist_ps, lhsT=ones, rhs=hm,
                    start=(tile_idx == 0), stop=(tile_idx == N_TILES - 1),
                )

        if hist_ps is not None:
            hist_sb = out_pool.tile([1, 2 * E], F32, tag="hist")
            nc.vector.tensor_copy(out=hist_sb, in_=hist_ps)
            nc.scalar.dma_start(out=hist_d, in_=hist_sb)


def build_program(repeats=1, stage="full", burst=True):
    nc = bacc.Bacc(
        "TRN2", target_bir_lowering=False, debug=False, num_devices=N_CORES
    )
    x_d = nc.dram_tensor("x", [T_CORE, H], F32, kind="ExternalInput").ap()
    wt_d = nc.dram_tensor("wt", [P, NH * E], F32, kind="ExternalInput").ap()
    probs_d = nc.dram_tensor("probs", [T_CORE, E], F32, kind="ExternalOutput").ap()
    topw_d = nc.dram_tensor("topw", [T_CORE, TOPK], F32, kind="ExternalOutput").ap()
    topi_d = nc.dram_tensor("topi", [T_CORE, TOPK], U32, kind="ExternalOutput").ap()
    hist_d = nc.dram_tensor("hist", [1, 2 * E], F32, kind="ExternalOutput").ap()

    with tile.TileContext(nc) as tc:
        _emit(tc, x_d, wt_d, probs_d, topw_d, topi_d, hist_d, repeats=repeats,
              stage=stage, burst=burst)
    # Bacc compile legalizes sync waits (>=2 waits per instruction are split
    # into InstEventSemaphore; walrus only encodes one wait per TPB inst).
    nc.compile()
    return nc


def shard_inputs(hidden_states, gate_weight):
    """Build per-core input maps from the full inputs."""
    x = np.ascontiguousarray(
        np.asarray(hidden_states, dtype=np.float32).reshape(T_TOTAL, H)
    )
    w = np.asarray(gate_weight, dtype=np.float32)
    # W^T in H-chunk-blocked layout: wtb[p, c*E + e] = W[e, c*128 + p]
    wtb = np.ascontiguousarray(
        w.T.reshape(NH, P, E).transpose(1, 0, 2).reshape(P, NH * E)
    )
    in_maps = []
    for c in range(N_CORES):
        shard = np.ascontiguousarray(x[c * T_CORE:(c + 1) * T_CORE])
        in_maps.append({"x": shard, "wt": wtb})
    return in_maps


def assemble_outputs(results):
    """Combine per-core result dicts into the reference's output structure."""
    probs = np.concatenate([r["probs"] for r in results], axis=0)
    topw = np.concatenate([r["topw"] for r in results], axis=0)
    topi = np.concatenate([r["topi"] for r in results], axis=0)
    hist = np.sum(np.stack([r["hist"][0] for r in results]).astype(np.float32),
                  axis=0, dtype=np.float32)
    ew = hist[:E]
    ec = hist[E:]
    loss = np.float32(np.sum(ew * ec, dtype=np.float32) / (EXPECTED_LOAD ** 2))

    top_k_weights = topw.reshape(B, S, TOPK)
    routing_probs = probs.reshape(B, S, E)
    top_k_indices = topi.view(np.int32).reshape(B, S, TOPK)
    return top_k_weights, routing_probs, loss, top_k_indices


_PROGRAM = None


def _get_program():
    global _PROGRAM
    if _PROGRAM is None:
        _PROGRAM = build_program()
    return _PROGRAM


def run(hidden_states, gate_weight, trace=False):
    nc = _get_program()
    in_maps = shard_inputs(hidden_states, gate_weight)
    res = run_bass_kernel_spmd(
        nc, in_maps, list(range(N_CORES)), trace=trace
    )
    return assemble_outputs(res.results), res


def kernel(hidden_states, gate_weight):
    outputs, _ = run(hidden_states, gate_weight)
    return outputs
